# revision 1
# baseline (speedup 1.0000x reference)
"""ContextualAttention Trainium2 kernel.

Full (unsharded) inputs in, full output out. Internally shards across 8
NeuronCores as batch x head-group: core c -> batch c//2, heads
(c%2)*8 .. (c%2)*8+8.  Each core computes a partial output projection for
its batch; the host sums the two partials per batch and adds the bias.

Per-core dataflow (per-core shapes; E=1024, heads=8 local, d=64):
  xcT [E, 2048]  (pre-transposed on host)
  QT = WqT-slices.T @ xcT-slices   -> [512, 1024]  (head dim on partitions)
  KT -> [512, 2048], V natural -> [2048, 512] (+ones col per head)
  QK-layernorm over head dim (=partitions) via block-ones matmul trick,
  gamma/beta and the 1/E**0.25 scaling folded in.
  S^T tiles = K_hT.T @ Q_hT  (two heads packed in the PE via row tiling)
  P = exp(S^T)  (no max subtraction: |scores| <= 2 after LN)
  [out_h^T; denom] = [V_h | 1].T @ P  accumulated over j tiles
  out_h^T *= 1/denom (broadcast via DMA)
  y_part = outT.T @ WuT-slices  -> [1024, 1024]
"""

import sys

import numpy as np

sys.path.insert(0, "/opt/trn_rl_repo")

import concourse.bass as bass  # noqa: E402
from concourse import bacc  # noqa: E402
import concourse.tile as tile  # noqa: E402
from concourse import mybir  # noqa: E402
from concourse.bass_utils import run_bass_kernel_spmd  # noqa: E402

F32 = mybir.dt.float32
F32R = mybir.dt.float32r
AF = mybir.ActivationFunctionType

E = 1024
TI = 1024
T = 2048
HL = 8  # heads per core
D = 64  # head size
DL = HL * D  # 512, local q/k/v dim
B = 4
SCALE = float(E) ** 0.25
EPS = 1e-5

# matmul-input dtype (float32r = relaxed-precision fp32, 4x faster matmuls).
# The BIR verifier requires every producer of an f32r-matmul input to emit
# f32r, so all matmul-feeding tensors are declared MMD end to end.
MMD = F32


def _mt(ap, dt):  # kept for selective overrides
    return ap.bitcast(dt) if ap.dtype != dt else ap


def _emit(tc, xcT, wqT, wkT, wvT, wuT, lno, lno2, gq, bq, gk, bk, y):
    nc = tc.nc
    from contextlib import ExitStack

    with ExitStack() as ctx:
        consts = ctx.enter_context(tc.tile_pool(name="consts", bufs=1))
        resid = ctx.enter_context(tc.tile_pool(name="resid", bufs=1))

        # ---- constants ----
        lno_sb = consts.tile([128, 128], MMD, tag="lno")
        nc.sync.dma_start(out=lno_sb, in_=lno)
        lno2_sb = consts.tile([128, 128], MMD, tag="lno2")
        nc.sync.dma_start(out=lno2_sb, in_=lno2)
        g_sb = {}
        b_sb = {}
        for name, src in (("q", gq), ("k", gk)):
            g_sb[name] = consts.tile([128, 1], F32, tag=f"g{name}", name=f"g{name}")
            nc.sync.dma_start(out=g_sb[name], in_=src)
        for name, src in (("q", bq), ("k", bk)):
            b_sb[name] = consts.tile([128, 1], F32, tag=f"b{name}", name=f"b{name}")
            nc.sync.dma_start(out=b_sb[name], in_=src)
        eps_sb = consts.tile([128, 1], F32, tag="eps")
        nc.vector.memset(eps_sb, EPS)

        # ---- residents (live through all phases) ----
        qt_sb = resid.tile([128, 4, TI], MMD, tag="qt")  # [2-head pair, chunk, i]
        kt_sb = resid.tile([128, 4, T], MMD, tag="kt")
        v_sb = resid.tile([128, 16, HL * (D + 1)], MMD, tag="v")  # +ones col

        # ones columns of v (col 64 of each head slot)
        v_heads = v_sb.rearrange("p j (h e) -> p j h e", e=D + 1)
        nc.vector.memset(v_heads[:, :, :, D : D + 1], 1.0)

        # ---- phase P: projections + LN (xc streamed in 512-token blocks) ----
        with (
            tc.tile_pool(name="xc", bufs=2) as xc_pool,
            tc.tile_pool(name="w", bufs=1) as w_pool,
            tc.tile_pool(name="ln_tmp", bufs=2) as ln_tmp,
            tc.tile_pool(name="pp", bufs=2, space="PSUM") as pp,
            tc.tile_pool(name="pstat", bufs=2, space="PSUM") as pstat,
        ):
            w_t = {}
            for wname, wT in (("q", wqT), ("k", wkT), ("v", wvT)):
                w_t[wname] = w_pool.tile(
                    [128, 8, DL], MMD, tag=f"w{wname}", name=f"w{wname}"
                )
                wT_r = wT.rearrange("(ko p) m -> p ko m", p=128)
                for k in range(8):
                    nc.sync.dma_start(out=w_t[wname][:, k], in_=wT_r[:, k])

            xcT_r = xcT.rearrange("(ko p) t -> p ko t", p=128)

            def ln_store(ps, dest, gs, bs):
                # layernorm over the 64-partition head blocks of psum tile ps.
                # centering via one matmul with (I - blockones/64); variance
                # via a second block-ones matmul on Square(centered).
                raw = ln_tmp.tile([128, 512], MMD, tag="raw", name="raw")
                nc.vector.tensor_copy(raw, ps)
                cen = pstat.tile([128, 512], F32, tag="cen", name="cen")
                nc.tensor.matmul(
                    cen, lhsT=lno2_sb, rhs=raw, start=True, stop=True,
                )
                sqc = ln_tmp.tile([128, 512], MMD, tag="sqc", name="sqc")
                nc.scalar.activation(sqc, cen, AF.Square)
                mvar = pstat.tile([128, 512], F32, tag="mvar", name="mvar")
                nc.tensor.matmul(
                    mvar, lhsT=lno_sb, rhs=sqc, start=True, stop=True,
                )
                a = ln_tmp.tile([128, 512], F32, tag="a", name="a")
                nc.scalar.activation(a, mvar, AF.Sqrt, bias=eps_sb)
                nc.vector.reciprocal(a, a)
                nc.vector.tensor_scalar_mul(a, a, gs)
                nc.vector.tensor_mul(dest, cen, a)
                nc.vector.tensor_scalar_add(dest, dest, bs)

            for nt in range(4):  # 512-token blocks of xc
                xc_nt = xc_pool.tile([128, 8, 512], MMD, tag="xc", name="xc_nt")
                for k in range(8):
                    nc.sync.dma_start(
                        out=xc_nt[:, k], in_=xcT_r[:, k, nt * 512 : (nt + 1) * 512]
                    )
                for mc in range(4):
                    # Q projection covers only the first TI tokens
                    projs = [("k", kt_sb)] if nt >= 2 else [("q", qt_sb), ("k", kt_sb)]
                    for wname, dest in projs:
                        ps = pp.tile([128, 512], F32, tag="pp", name="ps")
                        for k in range(8):
                            nc.tensor.matmul(
                                ps,
                                lhsT=w_t[wname][:, k, mc * 128 : (mc + 1) * 128],
                                rhs=xc_nt[:, k],
                                start=(k == 0),
                                stop=(k == 7),
                            )
                        ln_store(
                            ps,
                            dest[:, mc, nt * 512 : (nt + 1) * 512],
                            g_sb[wname],
                            b_sb[wname],
                        )
                for tl in range(4):  # V natural projection, 128-token tiles
                    tt = nt * 4 + tl
                    ps = pp.tile([128, 512], F32, tag="pp", name="ps")
                    for k in range(8):
                        nc.tensor.matmul(
                            ps,
                            lhsT=xc_nt[:, k, tl * 128 : (tl + 1) * 128],
                            rhs=w_t["v"][:, k, :],
                            start=(k == 0),
                            stop=(k == 7),
                        )
                    nc.scalar.activation(
                        v_heads[:, tt, :, 0:D],
                        ps.rearrange("p (h e) -> p h e", e=D),
                        AF.Copy,
                    )

        # ---- phase A: attention ----
        resid2 = ctx.enter_context(tc.tile_pool(name="resid2", bufs=1))
        ot_sb = resid2.tile([128, 4, TI], MMD, tag="ot")
        wu_sb = resid2.tile([128, 4, E], MMD, tag="wu")
        wuT_r = wuT.rearrange("(ko p) e -> p ko e", p=128)
        for k in range(4):
            nc.sync.dma_start(out=wu_sb[:, k], in_=wuT_r[:, k])

        with (
            tc.tile_pool(name="pt", bufs=4) as pt_pool,
            tc.tile_pool(name="sm", bufs=2) as sm_pool,
            tc.tile_pool(name="dr", bufs=2, space="DRAM") as dr_pool,
            tc.tile_pool(name="pqk", bufs=2, space="PSUM") as pqk,
            tc.tile_pool(name="ppv", bufs=1, space="PSUM") as ppv,
        ):
            for hp in range(4):
                for ic in range(2):
                    isl = slice(ic * 512, (ic + 1) * 512)
                    pv = [
                        ppv.tile([D + 1, 512], F32, tag=f"pv{par}", name=f"pv{par}")
                        for par in (0, 1)
                    ]
                    for jt in range(16):
                        jsl = slice(jt * 128, (jt + 1) * 128)
                        for par in (0, 1):
                            prt = slice(par * 64, par * 64 + 64)
                            qk = pqk.tile([128, 512], F32, tag=f"qk{par}", name=f"qk{par}")
                            nc.tensor.matmul(
                                qk,
                                lhsT=kt_sb[prt, hp, jsl],
                                rhs=qt_sb[prt, hp, isl],
                                start=True,
                                stop=True,
                            )
                            pt = pt_pool.tile([128, 512], MMD, tag=f"pt{par}", name=f"pt{par}")
                            nc.scalar.activation(pt, qk, AF.Exp)
                            h = 2 * hp + par
                            nc.tensor.matmul(
                                pv[par],
                                lhsT=v_sb[:, jt, h * (D + 1) : (h + 1) * (D + 1)],
                                rhs=pt,
                                start=(jt == 0),
                                stop=(jt == 15),
                            )
                    for par in (0, 1):
                        rc = sm_pool.tile([D + 1, 512], F32, tag=f"rc{par}", name=f"rc{par}")
                        nc.vector.reciprocal(rc[D : D + 1, :], pv[par][D : D + 1, :])
                        dt = dr_pool.tile([1, 512], F32, tag=f"dr{par}", name=f"dr{par}")
                        nc.sync.dma_start(out=dt, in_=rc[D : D + 1, :])
                        bc = sm_pool.tile([64, 512], F32, tag=f"bc{par}", name=f"bc{par}")
                        bcast_src = bass.AP(
                            tensor=dt.tensor, offset=dt.offset, ap=[[0, 64], [1, 512]]
                        )
                        nc.gpsimd.dma_start(out=bc, in_=bcast_src)
                        if par == 0:
                            nc.vector.tensor_mul(
                                ot_sb[0:64, hp, isl], pv[par][0:D, :], bc
                            )
                        else:
                            tmp = sm_pool.tile([64, 512], MMD, tag="tmpB", name="tmpB")
                            nc.vector.tensor_mul(tmp, pv[par][0:D, :], bc)
                            nc.sync.dma_start(out=ot_sb[64:128, hp, isl], in_=tmp)

        # ---- phase U: unify ----
        with (
            tc.tile_pool(name="yp", bufs=3) as y_pool,
            tc.tile_pool(name="pu", bufs=4, space="PSUM") as pu,
        ):
            for it in range(8):
                for et in range(2):
                    py = pu.tile([128, 512], F32, tag="py", name="py")
                    for hp in range(4):
                        nc.tensor.matmul(
                            py,
                            lhsT=ot_sb[:, hp, it * 128 : (it + 1) * 128],
                            rhs=wu_sb[:, hp, et * 512 : (et + 1) * 512],
                            start=(hp == 0),
                            stop=(hp == 3),
                        )
                    ysb = y_pool.tile([128, 512], F32, tag="y", name="ysb")
                    nc.scalar.activation(ysb, py, AF.Copy)
                    nc.sync.dma_start(
                        out=y[it * 128 : (it + 1) * 128, et * 512 : (et + 1) * 512],
                        in_=ysb,
                    )


_NC_CACHE = None


def build_nc():
    global _NC_CACHE
    if _NC_CACHE is not None:
        return _NC_CACHE
    nc = bacc.Bacc(
        trn_type="TRN2",
        target_bir_lowering=False,
        debug=False,
        enable_asserts=False,
        num_devices=8,
    )
    xcT = nc.dram_tensor("xcT", [E, T], MMD, kind="ExternalInput").ap()
    wqT = nc.dram_tensor("wqT", [E, DL], MMD, kind="ExternalInput").ap()
    wkT = nc.dram_tensor("wkT", [E, DL], MMD, kind="ExternalInput").ap()
    wvT = nc.dram_tensor("wvT", [E, DL], MMD, kind="ExternalInput").ap()
    wuT = nc.dram_tensor("wuT", [DL, E], MMD, kind="ExternalInput").ap()
    lno = nc.dram_tensor("lno", [128, 128], MMD, kind="ExternalInput").ap()
    lno2 = nc.dram_tensor("lno2", [128, 128], MMD, kind="ExternalInput").ap()
    gq = nc.dram_tensor("gq", [128, 1], F32, kind="ExternalInput").ap()
    bq = nc.dram_tensor("bq", [128, 1], F32, kind="ExternalInput").ap()
    gk = nc.dram_tensor("gk", [128, 1], F32, kind="ExternalInput").ap()
    bk = nc.dram_tensor("bk", [128, 1], F32, kind="ExternalInput").ap()
    y = nc.dram_tensor("y", [TI, E], F32, kind="ExternalOutput").ap()
    with tile.TileContext(nc) as tc:
        _emit(tc, xcT, wqT, wkT, wvT, wuT, lno, lno2, gq, bq, gk, bk, y)
    nc.compile()
    _NC_CACHE = nc
    return nc


def _prep_inputs(x, context, Wq, Wk, Wv, Wu, q_gamma, q_beta, k_gamma, k_beta):
    f = lambda a: np.ascontiguousarray(np.asarray(a, dtype=np.float32))
    x, context = f(x), f(context)
    Wq, Wk, Wv, Wu = f(Wq), f(Wk), f(Wv), f(Wu)
    lno = np.kron(np.eye(2, dtype=np.float32), np.ones((D, D), np.float32)) / D
    consts = {
        "lno": lno,
        "lno2": np.eye(128, dtype=np.float32) - lno,
        "gq": f(np.tile(q_gamma, 2) / SCALE)[:, None],
        "bq": f(np.tile(q_beta, 2) / SCALE)[:, None],
        "gk": f(np.tile(k_gamma, 2) / SCALE)[:, None],
        "bk": f(np.tile(k_beta, 2) / SCALE)[:, None],
    }
    in_maps = []
    for c in range(8):
        b, hh = c // 2, c % 2
        sl = slice(hh * DL, (hh + 1) * DL)
        xc = np.concatenate([x[b], context[b]], axis=0)
        in_maps.append(
            {
                "xcT": np.ascontiguousarray(xc.T),
                "wqT": np.ascontiguousarray(Wq[sl].T),
                "wkT": np.ascontiguousarray(Wk[sl].T),
                "wvT": np.ascontiguousarray(Wv[sl].T),
                "wuT": np.ascontiguousarray(Wu[:, sl].T),
                **consts,
            }
        )
    return in_maps


def run(inputs, trace=False):
    bu = np.asarray(inputs["bu"], dtype=np.float32)
    in_maps = _prep_inputs(
        inputs["x"], inputs["context"], inputs["Wq"], inputs["Wk"], inputs["Wv"],
        inputs["Wu"], inputs["q_gamma"], inputs["q_beta"], inputs["k_gamma"],
        inputs["k_beta"],
    )
    nc = build_nc()
    res = run_bass_kernel_spmd(nc, in_maps, list(range(8)), trace=trace)
    y = np.empty((B, TI, E), dtype=np.float32)
    for b in range(B):
        y[b] = res.results[2 * b]["y"] + res.results[2 * b + 1]["y"] + bu
    return y, res.exec_time_ns


def kernel(**inputs):
    y, _ = run(inputs, trace=False)
    return y



# revision 3
# speedup vs baseline: 1.7892x; 1.7892x over previous
"""ContextualAttention Trainium2 kernel.

Full (unsharded) inputs in, full output out. Internally shards across 8
NeuronCores as batch x head-group: core c -> batch c//2, heads
(c%2)*8 .. (c%2)*8+8.  Each core computes a partial output projection for
its batch; the host sums the two partials per batch and adds the bias.

Per-core dataflow (per-core shapes; E=1024, heads=8 local, d=64):
  xcT [E, 2048]  (pre-transposed on host)
  QT = WqT-slices.T @ xcT-slices   -> [512, 1024]  (head dim on partitions)
  KT -> [512, 2048], V natural -> [2048, 512] (+ones col per head)
  QK-layernorm over head dim (=partitions) via block-ones matmul trick,
  gamma/beta and the 1/E**0.25 scaling folded in.
  S^T tiles = K_hT.T @ Q_hT  (two heads packed in the PE via row tiling)
  P = exp(S^T)  (no max subtraction: |scores| <= 2 after LN)
  [out_h^T; denom] = [V_h | 1].T @ P  accumulated over j tiles
  out_h^T *= 1/denom (broadcast via DMA)
  y_part = outT.T @ WuT-slices  -> [1024, 1024]
"""

import sys

import numpy as np

sys.path.insert(0, "/opt/trn_rl_repo")

import concourse.bass as bass  # noqa: E402
from concourse import bacc  # noqa: E402
import concourse.tile as tile  # noqa: E402
from concourse import mybir  # noqa: E402
from concourse.bass_utils import run_bass_kernel_spmd  # noqa: E402

F32 = mybir.dt.float32
F32R = mybir.dt.float32r
AF = mybir.ActivationFunctionType

E = 1024
TI = 1024
T = 2048
HL = 8  # heads per core
D = 64  # head size
DL = HL * D  # 512, local q/k/v dim
B = 4
SCALE = float(E) ** 0.25
EPS = 1e-5

# matmul-input dtype (float32r = relaxed-precision fp32, 4x faster matmuls).
# The BIR verifier requires every producer of an f32r-matmul input to emit
# f32r, so all matmul-feeding tensors are declared MMD end to end.
MMD = F32R


def _mt(ap, dt):  # kept for selective overrides
    return ap.bitcast(dt) if ap.dtype != dt else ap


def _emit(tc, xcT, wqT, wkT, wvT, wuT, lno, lno2, gq, bq, gk, bk, y):
    nc = tc.nc
    from contextlib import ExitStack

    with ExitStack() as ctx:
        consts = ctx.enter_context(tc.tile_pool(name="consts", bufs=1))
        resid = ctx.enter_context(tc.tile_pool(name="resid", bufs=1))

        # ---- constants ----
        lno_sb = consts.tile([128, 128], MMD, tag="lno")
        nc.sync.dma_start(out=lno_sb, in_=lno)
        lno2_sb = consts.tile([128, 128], MMD, tag="lno2")
        nc.sync.dma_start(out=lno2_sb, in_=lno2)
        g_sb = {}
        b_sb = {}
        for name, src in (("q", gq), ("k", gk)):
            g_sb[name] = consts.tile([128, 1], F32, tag=f"g{name}", name=f"g{name}")
            nc.sync.dma_start(out=g_sb[name], in_=src)
        for name, src in (("q", bq), ("k", bk)):
            b_sb[name] = consts.tile([128, 1], F32, tag=f"b{name}", name=f"b{name}")
            nc.sync.dma_start(out=b_sb[name], in_=src)
        eps_sb = consts.tile([128, 1], F32, tag="eps")
        nc.vector.memset(eps_sb, EPS)

        # ---- residents (live through all phases) ----
        qt_sb = resid.tile([128, 4, TI], MMD, tag="qt")  # [2-head pair, chunk, i]
        kt_sb = resid.tile([128, 4, T], MMD, tag="kt")
        v_sb = resid.tile([128, 16, HL * (D + 1)], MMD, tag="v")  # +ones col

        # ones columns of v (col 64 of each head slot)
        v_heads = v_sb.rearrange("p j (h e) -> p j h e", e=D + 1)
        nc.vector.memset(_mt(v_heads[:, :, :, D : D + 1], F32), 1.0)

        # ---- phase P: projections + LN (xc streamed in 512-token blocks) ----
        with (
            tc.tile_pool(name="xc", bufs=2) as xc_pool,
            tc.tile_pool(name="w", bufs=1) as w_pool,
            tc.tile_pool(name="ln_tmp", bufs=2) as ln_tmp,
            tc.tile_pool(name="pp", bufs=2, space="PSUM") as pp,
            tc.tile_pool(name="pstat", bufs=2, space="PSUM") as pstat,
        ):
            w_t = {}
            for wname, wT in (("q", wqT), ("k", wkT), ("v", wvT)):
                w_t[wname] = w_pool.tile(
                    [128, 8, DL], MMD, tag=f"w{wname}", name=f"w{wname}"
                )
                wT_r = wT.rearrange("(ko p) m -> p ko m", p=128)
                for k in range(8):
                    nc.sync.dma_start(out=w_t[wname][:, k], in_=wT_r[:, k])

            xcT_r = xcT.rearrange("(ko p) t -> p ko t", p=128)

            def ln_store(ps, dest, gs, bs):
                # layernorm over the 64-partition head blocks of psum tile ps.
                # centering via one matmul with (I - blockones/64); variance
                # via a second block-ones matmul on Square(centered).
                raw = ln_tmp.tile([128, 512], MMD, tag="raw", name="raw")
                nc.vector.tensor_copy(raw, ps)
                cen = pstat.tile([128, 512], F32, tag="cen", name="cen")
                nc.tensor.matmul(
                    cen, lhsT=lno2_sb, rhs=raw, start=True, stop=True,
                )
                sqc = ln_tmp.tile([128, 512], MMD, tag="sqc", name="sqc")
                nc.scalar.activation(sqc, cen, AF.Square)
                mvar = pstat.tile([128, 512], F32, tag="mvar", name="mvar")
                nc.tensor.matmul(
                    mvar, lhsT=lno_sb, rhs=sqc, start=True, stop=True,
                )
                a = ln_tmp.tile([128, 512], F32, tag="a", name="a")
                nc.scalar.activation(a, mvar, AF.Sqrt, bias=eps_sb)
                nc.vector.reciprocal(a, a)
                nc.vector.tensor_scalar_mul(a, a, gs)
                nc.vector.tensor_mul(dest, cen, a)
                nc.vector.tensor_scalar_add(dest, dest, bs)

            for nt in range(4):  # 512-token blocks of xc
                xc_nt = xc_pool.tile([128, 8, 512], MMD, tag="xc", name="xc_nt")
                for k in range(8):
                    nc.sync.dma_start(
                        out=xc_nt[:, k], in_=xcT_r[:, k, nt * 512 : (nt + 1) * 512]
                    )
                for mc in range(4):
                    # Q projection covers only the first TI tokens
                    projs = [("k", kt_sb)] if nt >= 2 else [("q", qt_sb), ("k", kt_sb)]
                    for wname, dest in projs:
                        ps = pp.tile([128, 512], F32, tag="pp", name="ps")
                        for k in range(8):
                            nc.tensor.matmul(
                                ps,
                                lhsT=w_t[wname][:, k, mc * 128 : (mc + 1) * 128],
                                rhs=xc_nt[:, k],
                                start=(k == 0),
                                stop=(k == 7),
                            )
                        ln_store(
                            ps,
                            dest[:, mc, nt * 512 : (nt + 1) * 512],
                            g_sb[wname],
                            b_sb[wname],
                        )
                for tl in range(4):  # V natural projection, 128-token tiles
                    tt = nt * 4 + tl
                    ps = pp.tile([128, 512], F32, tag="pp", name="ps")
                    for k in range(8):
                        nc.tensor.matmul(
                            ps,
                            lhsT=xc_nt[:, k, tl * 128 : (tl + 1) * 128],
                            rhs=w_t["v"][:, k, :],
                            start=(k == 0),
                            stop=(k == 7),
                        )
                    nc.scalar.activation(
                        v_heads[:, tt, :, 0:D],
                        ps.rearrange("p (h e) -> p h e", e=D),
                        AF.Copy,
                    )

        # ---- phase A: attention ----
        resid2 = ctx.enter_context(tc.tile_pool(name="resid2", bufs=1))
        ot_sb = resid2.tile([128, 4, TI], MMD, tag="ot")
        wu_sb = resid2.tile([128, 4, E], MMD, tag="wu")
        wuT_r = wuT.rearrange("(ko p) e -> p ko e", p=128)
        for k in range(4):
            nc.sync.dma_start(out=wu_sb[:, k], in_=wuT_r[:, k])

        with (
            tc.tile_pool(name="pt", bufs=4) as pt_pool,
            tc.tile_pool(name="sm", bufs=2) as sm_pool,
            tc.tile_pool(name="dr", bufs=2, space="DRAM") as dr_pool,
            tc.tile_pool(name="pqk", bufs=2, space="PSUM") as pqk,
            tc.tile_pool(name="ppv", bufs=1, space="PSUM") as ppv,
        ):
            for hp in range(4):
                for ic in range(2):
                    isl = slice(ic * 512, (ic + 1) * 512)
                    pv = [
                        ppv.tile([D + 1, 512], F32, tag=f"pv{par}", name=f"pv{par}")
                        for par in (0, 1)
                    ]
                    for jt in range(16):
                        jsl = slice(jt * 128, (jt + 1) * 128)
                        for par in (0, 1):
                            prt = slice(par * 64, par * 64 + 64)
                            qk = pqk.tile([128, 512], F32, tag=f"qk{par}", name=f"qk{par}")
                            nc.tensor.matmul(
                                qk,
                                lhsT=kt_sb[prt, hp, jsl],
                                rhs=qt_sb[prt, hp, isl],
                                start=True,
                                stop=True,
                            )
                            pt = pt_pool.tile([128, 512], MMD, tag=f"pt{par}", name=f"pt{par}")
                            nc.scalar.activation(pt, qk, AF.Exp)
                            h = 2 * hp + par
                            nc.tensor.matmul(
                                pv[par],
                                lhsT=v_sb[:, jt, h * (D + 1) : (h + 1) * (D + 1)],
                                rhs=pt,
                                start=(jt == 0),
                                stop=(jt == 15),
                            )
                    for par in (0, 1):
                        rc = sm_pool.tile([D + 1, 512], F32, tag=f"rc{par}", name=f"rc{par}")
                        nc.vector.reciprocal(rc[D : D + 1, :], pv[par][D : D + 1, :])
                        dt = dr_pool.tile([1, 512], F32, tag=f"dr{par}", name=f"dr{par}")
                        nc.sync.dma_start(out=dt, in_=rc[D : D + 1, :])
                        bc = sm_pool.tile([64, 512], F32, tag=f"bc{par}", name=f"bc{par}")
                        bcast_src = bass.AP(
                            tensor=dt.tensor, offset=dt.offset, ap=[[0, 64], [1, 512]]
                        )
                        nc.gpsimd.dma_start(out=bc, in_=bcast_src)
                        if par == 0:
                            nc.vector.tensor_mul(
                                ot_sb[0:64, hp, isl], pv[par][0:D, :], bc
                            )
                        else:
                            tmp = sm_pool.tile([64, 512], MMD, tag="tmpB", name="tmpB")
                            nc.vector.tensor_mul(tmp, pv[par][0:D, :], bc)
                            nc.sync.dma_start(out=ot_sb[64:128, hp, isl], in_=tmp)

        # ---- phase U: unify ----
        with (
            tc.tile_pool(name="yp", bufs=3) as y_pool,
            tc.tile_pool(name="pu", bufs=4, space="PSUM") as pu,
        ):
            for it in range(8):
                for et in range(2):
                    py = pu.tile([128, 512], F32, tag="py", name="py")
                    for hp in range(4):
                        nc.tensor.matmul(
                            py,
                            lhsT=ot_sb[:, hp, it * 128 : (it + 1) * 128],
                            rhs=wu_sb[:, hp, et * 512 : (et + 1) * 512],
                            start=(hp == 0),
                            stop=(hp == 3),
                        )
                    ysb = y_pool.tile([128, 512], F32, tag="y", name="ysb")
                    nc.scalar.activation(ysb, py, AF.Copy)
                    nc.sync.dma_start(
                        out=y[it * 128 : (it + 1) * 128, et * 512 : (et + 1) * 512],
                        in_=ysb,
                    )


_NC_CACHE = None


def build_nc():
    global _NC_CACHE
    if _NC_CACHE is not None:
        return _NC_CACHE
    nc = bacc.Bacc(
        trn_type="TRN2",
        target_bir_lowering=False,
        debug=False,
        enable_asserts=False,
        num_devices=8,
    )
    xcT = nc.dram_tensor("xcT", [E, T], MMD, kind="ExternalInput").ap()
    wqT = nc.dram_tensor("wqT", [E, DL], MMD, kind="ExternalInput").ap()
    wkT = nc.dram_tensor("wkT", [E, DL], MMD, kind="ExternalInput").ap()
    wvT = nc.dram_tensor("wvT", [E, DL], MMD, kind="ExternalInput").ap()
    wuT = nc.dram_tensor("wuT", [DL, E], MMD, kind="ExternalInput").ap()
    lno = nc.dram_tensor("lno", [128, 128], MMD, kind="ExternalInput").ap()
    lno2 = nc.dram_tensor("lno2", [128, 128], MMD, kind="ExternalInput").ap()
    gq = nc.dram_tensor("gq", [128, 1], F32, kind="ExternalInput").ap()
    bq = nc.dram_tensor("bq", [128, 1], F32, kind="ExternalInput").ap()
    gk = nc.dram_tensor("gk", [128, 1], F32, kind="ExternalInput").ap()
    bk = nc.dram_tensor("bk", [128, 1], F32, kind="ExternalInput").ap()
    y = nc.dram_tensor("y", [TI, E], F32, kind="ExternalOutput").ap()
    with tile.TileContext(nc) as tc:
        _emit(tc, xcT, wqT, wkT, wvT, wuT, lno, lno2, gq, bq, gk, bk, y)
    nc.compile()
    _NC_CACHE = nc
    return nc


def _prep_inputs(x, context, Wq, Wk, Wv, Wu, q_gamma, q_beta, k_gamma, k_beta):
    f = lambda a: np.ascontiguousarray(np.asarray(a, dtype=np.float32))
    x, context = f(x), f(context)
    Wq, Wk, Wv, Wu = f(Wq), f(Wk), f(Wv), f(Wu)
    lno = np.kron(np.eye(2, dtype=np.float32), np.ones((D, D), np.float32)) / D
    consts = {
        "lno": lno,
        "lno2": np.eye(128, dtype=np.float32) - lno,
        "gq": f(np.tile(q_gamma, 2) / SCALE)[:, None],
        "bq": f(np.tile(q_beta, 2) / SCALE)[:, None],
        "gk": f(np.tile(k_gamma, 2) / SCALE)[:, None],
        "bk": f(np.tile(k_beta, 2) / SCALE)[:, None],
    }
    in_maps = []
    for c in range(8):
        b, hh = c // 2, c % 2
        sl = slice(hh * DL, (hh + 1) * DL)
        xc = np.concatenate([x[b], context[b]], axis=0)
        in_maps.append(
            {
                "xcT": np.ascontiguousarray(xc.T),
                "wqT": np.ascontiguousarray(Wq[sl].T),
                "wkT": np.ascontiguousarray(Wk[sl].T),
                "wvT": np.ascontiguousarray(Wv[sl].T),
                "wuT": np.ascontiguousarray(Wu[:, sl].T),
                **consts,
            }
        )
    return in_maps


def run(inputs, trace=False):
    bu = np.asarray(inputs["bu"], dtype=np.float32)
    in_maps = _prep_inputs(
        inputs["x"], inputs["context"], inputs["Wq"], inputs["Wk"], inputs["Wv"],
        inputs["Wu"], inputs["q_gamma"], inputs["q_beta"], inputs["k_gamma"],
        inputs["k_beta"],
    )
    nc = build_nc()
    res = run_bass_kernel_spmd(nc, in_maps, list(range(8)), trace=trace)
    y = np.empty((B, TI, E), dtype=np.float32)
    for b in range(B):
        y[b] = res.results[2 * b]["y"] + res.results[2 * b + 1]["y"] + bu
    return y, res.exec_time_ns


def kernel(**inputs):
    y, _ = run(inputs, trace=False)
    return y

